# revision 2
# baseline (speedup 1.0000x reference)
"""Single-head attention on 8 Trainium2 NeuronCores.

Problem: x[8, 2048, 768], Wq/Wk/Wv[768, 64]+biases, mask[2048, 2048] int32
Output:  softmax(mask(Q K^T / 8)) V   -> [8, 2048, 64] f32

Sharding: data-parallel over batch — core b computes batch element b.

Per-core dataflow (all matmuls bf16 in / fp32 psum accumulate):
  host:  xT = x[b].T (w-major, partition-major relayout), Wqk = [Wq | Wk/8],
         mT = mask.T (consumption-major, 0/1 uint8)
  warm:  ~16 junk matmuls with no data deps run at body start so the PE HAM
         clock-gate flips to 2.4 GHz before the real work arrives (otherwise
         the first ~30us runs at 1.2 GHz).
  DMA:   xt on the SP HWDGE ring (first in line, full bandwidth), weights +
         biases on the ACT HWDGE ring (parallel issue), mask via GpSimd SWDGE
         as uint8 cast to bf16 in-flight (halves mask HBM traffic) in 8 1MB
         chunks that land in consumption order.
  QK:    QK[n,128] = xT.T @ Wqk + bqk (bias via tensor_scalar), cast bf16 ->
         QQ/KK duplicated across partition halves (row-tiled score matmuls)
  V:     V[k,64] = xT.T @ Wv (xt-chunk stationary); bias folded in the DVE
         psum->sbuf copy against a broadcast bv tile; ones column appended so
         the PV matmul also produces the softmax denominator for free
  ST:    ST[k,q] = KK.T @ QQ per 128-k-chunk (K=64 contraction: two chunks run
         concurrently in the PE array via row tiling at partitions 0/64)
  P:     P = exp(ST) on ScalarE (psum -> sbuf bf16), P *= mT (VectorE)
  OT:    OT[65,q] += V'[kchunk].T @ P[kchunk]  (accumulate over 16 k-chunks;
         k outer so each mask chunk is consumed right when it lands)
  out:   OT[65,2048] copied psum->sbuf (VectorE; ScalarE stays exp-only) and
         DMA'd; host does the final out[q,h] = OT[h,q]/OT[64,q] normalization.
"""

import numpy as np
import ml_dtypes

import bass_rust
import concourse.bass as bass
import concourse.mybir as mybir
import concourse.tile as tile
from concourse.bass_utils import run_bass_kernel_spmd

BF16 = ml_dtypes.bfloat16
F32 = mybir.dt.float32
BF = mybir.dt.bfloat16
U8 = mybir.dt.uint8

N_CORES = 8
SEQ = 2048
WIDTH = 768
HEAD = 64
NCH = WIDTH // 128      # 6 contraction chunks for the projections
NKC = SEQ // 128        # 16 key chunks
QT = 1024               # q tile (columns processed per main-loop sweep)
NQT = SEQ // QT
NMC = 8                 # mask DMA chunks (1 MiB bf16 each, consumption order)
N_WARM = 16             # junk matmuls to flip the HAM clock gate early


def _split_excess_waits(nc, max_waits=1):
    """walrus in this container rejects >1 sync wait per instruction; hoist
    extras onto preceding same-engine NoOps (same semantics: the engine
    executes its stream in order, so waiting earlier is equivalent)."""
    n = 0
    for bb in nc.main_func.blocks:
        new_list = []
        for ins in bb.instructions:
            si = ins.sync_info
            if si is not None and len(si.on_wait) > max_waits:
                waits = list(si.on_wait)
                extra, keep = waits[:-max_waits], waits[-max_waits:]
                for j, w in enumerate(extra):
                    nop = bass_rust.InstNoOp(
                        name=f"{ins.name}-ws{j}", engine=ins.engine, ins=[], outs=[]
                    )
                    nop.sync_info = mybir.SyncInfo(on_wait=[w], on_update=[])
                    new_list.append(nop)
                    n += 1
                ins.sync_info = mybir.SyncInfo(
                    on_wait=keep, on_update=list(si.on_update)
                )
            new_list.append(ins)
        bb.instructions = new_list
    return n


def _strip_tail(nc):
    """Drop the NRT pseudo-sync ISA op and the second all-engine barrier that
    TileContext emits after the semaphore reset — ~4-5us of fixed tail. The
    final DMA-drain + first barrier + sem reset are kept, so re-execution of
    the NEFF still starts from clean semaphores."""
    for bb in nc.main_func.blocks:
        ins_list = list(bb.instructions)
        idx = None
        for i, ins in enumerate(ins_list):
            if getattr(ins, "is_reset_sema", False):
                idx = i
        if idx is not None and idx > len(ins_list) - 20:
            bb.instructions = ins_list[:idx + 1]
    return nc


def _build():
    nc = bass.Bass("TRN2", target_bir_lowering=False, debug=False,
                   num_devices=N_CORES)

    # partition-major host layouts: row p holds everything partition p needs,
    # so each DMA is 128 large contiguous descriptors.
    xT_d = nc.declare_dram_parameter("xT", [128, 4 * NCH * 512], BF, False).ap()
    wqk_d = nc.declare_dram_parameter("Wqk", [128, NCH * 128], BF, False).ap()
    wv_d = nc.declare_dram_parameter("Wv", [128, NCH * HEAD], BF, False).ap()
    bqk_d = nc.declare_dram_parameter("bqk", [128, 1], F32, False).ap()
    bvb_d = nc.declare_dram_parameter("bvb", [128, HEAD], BF, False).ap()
    mT_d = nc.declare_dram_parameter("mT", [128, NKC * SEQ], U8, False).ap()
    ot_d = nc.declare_dram_parameter("ot", [HEAD + 1, SEQ], F32, True).ap()

    EXP = mybir.ActivationFunctionType.Exp

    with tile.TileContext(nc) as tc:
        with (
            tc.tile_pool(name="const", bufs=1) as const,
            tc.tile_pool(name="pp", bufs=6) as ppool,
            tc.tile_pool(name="ep", bufs=2) as epool,
            tc.tile_pool(name="stp", bufs=3, space="PSUM") as stp,
            tc.tile_pool(name="otp", bufs=1, space="PSUM") as otp,
        ):
            # ---- PE warm-up: data-independent matmuls fill the PE queue so
            # the HAM clock gate sees ~3.4us of sustained activity and flips
            # to 2.4 GHz right as the first real matmuls arrive. Results are
            # discarded (the pool buffer is recycled by the projections).
            wsrc = const.tile([128, 256], BF)
            nc.vector.memset(wsrc, 0.0)
            warm_ps = stp.tile([128, 256], F32, tag="st", name="warm_ps")
            for i in range(N_WARM):
                nc.tensor.matmul(warm_ps, wsrc[:, 0:128], wsrc[:, :],
                                 start=True, stop=True)

            # ---- inputs into SBUF ----
            # xt first on the SP ring: the whole kernel is gated on group t0.
            xt = const.tile([128, 4, NCH, 512], BF)
            for t in range(4):
                nc.sync.dma_start(
                    out=xt[:, t, :, :],
                    in_=xT_d[:, t * NCH * 512:(t + 1) * NCH * 512],
                )
            # weights + biases on the ACT HWDGE ring (parallel with xt).
            wqk = const.tile([128, NCH, 128], BF)
            nc.scalar.dma_start(out=wqk, in_=wqk_d)
            bqk = const.tile([128, 1], F32)
            nc.scalar.dma_start(out=bqk, in_=bqk_d)
            wv = const.tile([128, NCH, HEAD], BF)
            nc.scalar.dma_start(out=wv, in_=wv_d)
            bvb = const.tile([128, HEAD], BF)
            nc.scalar.dma_start(out=bvb, in_=bvb_d)

            # mask on the GpSimd SWDGE ring: uint8 in HBM, cast to bf16 in
            # flight. Consumption-major layout (q-half major, then k-chunk)
            # so big chunks land exactly in the order the main loop eats them.
            mt = const.tile([128, NQT * NKC, QT], BF)
            mch = (NQT * NKC) // NMC
            for m in range(NMC):
                nc.gpsimd.dma_start(
                    out=mt[:, m * mch:(m + 1) * mch, :],
                    in_=mT_d[:, m * mch * QT:(m + 1) * mch * QT],
                )

            # ---- projections, one 512-column group per xT DMA so they
            # pipeline behind the xT arrival; groups t2/t3 are emitted
            # interleaved into the early main loop (their xT lands later) ----
            qktmp = const.tile([128, SEQ], BF)   # Q on parts 0:64, K on 64:128
            qq = const.tile([128, SEQ], BF)      # Q duplicated on both halves
            kk = const.tile([128, SEQ], BF)      # K duplicated on both halves
            vp = const.tile([128, NKC, HEAD + 1], BF)   # V' with ones column

            def proj_qk(t):
                qk_ps = stp.tile([128, 512], F32, tag="st", name=f"qk_ps{t}")
                for c in range(NCH):
                    nc.tensor.matmul(
                        qk_ps, wqk[:, c, :], xt[:, t, c, :],
                        start=(c == 0), stop=(c == NCH - 1),
                    )
                cols = slice(t * 512, (t + 1) * 512)
                nc.vector.tensor_scalar(
                    out=qktmp[:, cols], in0=qk_ps, scalar1=bqk[:, 0:1],
                    scalar2=None, op0=mybir.AluOpType.add,
                )
                nc.vector.tensor_copy(out=qq[0:64, cols], in_=qktmp[0:64, cols])
                nc.vector.tensor_copy(out=qq[64:128, cols], in_=qktmp[0:64, cols])
                nc.vector.tensor_copy(out=kk[0:64, cols], in_=qktmp[64:128, cols])
                nc.vector.tensor_copy(out=kk[64:128, cols], in_=qktmp[64:128, cols])

            def proj_v(t, jlo, jhi):
                v_ps = stp.tile([128, jhi - jlo, HEAD], F32, tag="st",
                                name=f"v_ps{t}_{jlo}")
                for j in range(jlo, jhi):
                    for c in range(NCH):
                        nc.tensor.matmul(
                            v_ps[:, j - jlo, :],
                            xt[:, t, c, j * 128:(j + 1) * 128],
                            wv[:, c, :], start=(c == 0), stop=(c == NCH - 1),
                        )
                lo, hi = 4 * t + jlo, 4 * t + jhi
                # psum -> sbuf with the bias folded in (bv broadcast tile);
                # the ones column gives the softmax denominator in the PV mm.
                for j in range(jlo, jhi):
                    nc.vector.tensor_add(
                        out=vp[:, 4 * t + j, 0:HEAD],
                        in0=v_ps[:, j - jlo, :], in1=bvb,
                    )
                nc.vector.memset(vp[:, lo:hi, HEAD:HEAD + 1], 1.0)

            proj_qk(0)
            proj_v(0, 0, 4)
            proj_qk(1)

            # ---- main loop (q outer): scores -> exp -> mask -> PV ----
            # PV for iteration kp-1 is emitted alongside the scores for kp
            # (as a list of matmul specs) so the PE never stalls mid-stream.
            # The first two iterations of q0 are split into 512-wide halves:
            # the h0 half depends only on xT group t0, so exp starts earlier
            # while t1 is still arriving.
            NKP = NKC // 2
            for q in range(NQT):
                ot_ps = otp.tile([HEAD + 1, QT], F32, tag="ot", name=f"ot_ps{q}")
                prev = []
                for kp in range(NKP + 1):
                    cur = []
                    if kp < NKP:
                        k0, k1 = 2 * kp, 2 * kp + 1
                        split = (q == 0 and kp < 2)
                        halves = ((0, 512), (512, 1024)) if split else ((0, QT),)
                        for lo, hi in halves:
                            w = hi - lo
                            st_a = stp.tile([128, w], F32, tag="st",
                                            name=f"st_a{q}_{kp}_{lo}")
                            st_b = stp.tile([128, w], F32, tag="st",
                                            name=f"st_b{q}_{kp}_{lo}")
                            for h in range(w // 512):
                                gq = slice(q * QT + lo + h * 512,
                                           q * QT + lo + (h + 1) * 512)
                                nc.tensor.matmul(
                                    st_a[:, h * 512:(h + 1) * 512],
                                    kk[0:64, k0 * 128:(k0 + 1) * 128],
                                    qq[0:64, gq], start=True, stop=True,
                                )
                                nc.tensor.matmul(
                                    st_b[:, h * 512:(h + 1) * 512],
                                    kk[64:128, k1 * 128:(k1 + 1) * 128],
                                    qq[64:128, gq], start=True, stop=True,
                                )
                            p_a = ppool.tile([128, w], BF, tag="p",
                                             name=f"p_a{q}_{kp}_{lo}")
                            p_b = ppool.tile([128, w], BF, tag="p",
                                             name=f"p_b{q}_{kp}_{lo}")
                            nc.scalar.activation(p_a, st_a, EXP)
                            nc.scalar.activation(p_b, st_b, EXP)
                            mos = slice(lo, hi)
                            nc.vector.tensor_mul(p_a, p_a, mt[:, q * NKC + k0, mos])
                            nc.vector.tensor_mul(p_b, p_b, mt[:, q * NKC + k1, mos])
                            for h in range(w // 512):
                                ohs = slice(lo + h * 512, lo + (h + 1) * 512)
                                phs = slice(h * 512, (h + 1) * 512)
                                cur.append((ohs, k0, p_a, phs))
                                cur.append((ohs, k1, p_b, phs))
                        if q == 0:
                            # feed remaining projection work into the PE
                            # stream in small wedges so ScalarE never starves
                            if kp == 0:
                                proj_v(1, 0, 2)
                            elif kp == 1:
                                proj_v(1, 2, 4)
                                proj_qk(2)
                            elif kp == 2:
                                proj_v(2, 0, 2)
                            elif kp == 3:
                                proj_v(2, 2, 4)
                                proj_qk(3)
                            elif kp == 4:
                                proj_v(3, 0, 2)
                            elif kp == 5:
                                proj_v(3, 2, 4)
                    for ohs, k, p, phs in prev:
                        nc.tensor.matmul(
                            ot_ps[:, ohs], vp[:, k, :], p[:, phs],
                            start=(k == 0), stop=(k == NKC - 1),
                        )
                    prev = cur

                # psum -> sbuf on VectorE (ScalarE stays exp-only), then DMA
                # out on the SP ring (idle once the xt groups are in).
                ot_sb = epool.tile([HEAD + 1, QT], F32, tag="osb", name=f"ot_sb{q}")
                nc.vector.tensor_copy(out=ot_sb, in_=ot_ps)
                nc.sync.dma_start(out=ot_d[:, q * QT:(q + 1) * QT], in_=ot_sb)

    _split_excess_waits(nc)
    _strip_tail(nc)
    return nc


_CACHE = {}


def _get_nc():
    if "nc" not in _CACHE:
        _CACHE["nc"] = _build()
    return _CACHE["nc"]


def _prep_in_maps(x, Wq, bq, Wk, bk, Wv, bv, mask):
    x = np.asarray(x, dtype=np.float32)
    Wqk = np.concatenate(
        [np.asarray(Wq, np.float32), np.asarray(Wk, np.float32) * 0.125], axis=1
    )
    # partition-major: row p holds [c0 cols | c1 cols | ...] for w = c*128+p
    Wqkh = np.ascontiguousarray(
        Wqk.reshape(NCH, 128, 128).transpose(1, 0, 2).reshape(128, NCH * 128)
    ).astype(BF16)
    Wvh = np.ascontiguousarray(
        np.asarray(Wv, np.float32).reshape(NCH, 128, HEAD)
        .transpose(1, 0, 2).reshape(128, NCH * HEAD)
    ).astype(BF16)
    bqk = np.concatenate(
        [np.asarray(bq, np.float32), np.asarray(bk, np.float32) * 0.125]
    ).astype(np.float32).reshape(128, 1)
    bvb = np.broadcast_to(
        np.asarray(bv, np.float32).astype(BF16).reshape(1, HEAD), (128, HEAD)
    ).copy()
    # mTh[p, (h*NKC+c)*QT + j] = mask[h*QT+j, c*128+p]: consumption-major
    # (q-half, then k-chunk) as uint8 — cast to bf16 happens in the DMA.
    mTh = np.ascontiguousarray(
        np.asarray(mask, np.uint8).T.reshape(NKC, 128, NQT, QT)
        .transpose(1, 2, 0, 3).reshape(128, NKC * SEQ)
    )
    in_maps = []
    for b in range(N_CORES):
        # xth[p, t, c, j] = x[b][t*512+j, c*128+p]
        xth = np.ascontiguousarray(
            x[b].reshape(4, 512, NCH, 128).transpose(3, 0, 2, 1)
            .reshape(128, 4 * NCH * 512)
        ).astype(BF16)
        in_maps.append({
            "xT": xth, "Wqk": Wqkh, "Wv": Wvh, "bqk": bqk, "bvb": bvb,
            "mT": mTh,
        })
    return in_maps


def _run(in_maps, trace=False, **kw):
    nc = _get_nc()
    return run_bass_kernel_spmd(nc, in_maps, list(range(N_CORES)), trace=trace, **kw)


def kernel(x, Wq, bq, Wk, bk, Wv, bv, mask):
    in_maps = _prep_in_maps(x, Wq, bq, Wk, bk, Wv, bv, mask)
    res = _run(in_maps)
    out = np.empty((N_CORES, SEQ, HEAD), np.float32)
    for b in range(N_CORES):
        ot = np.asarray(res.results[b]["ot"])          # [65, 2048] f32
        out[b] = (ot[:HEAD] / ot[HEAD:HEAD + 1]).T     # normalize + transpose
    return out


# revision 7
# speedup vs baseline: 1.2270x; 1.2270x over previous
"""Single-head attention on 8 Trainium2 NeuronCores.

Problem: x[8, 2048, 768], Wq/Wk/Wv[768, 64]+biases, mask[2048, 2048] int32
Output:  softmax(mask(Q K^T / 8)) V   -> [8, 2048, 64] f32

Sharding: data-parallel over batch — core b computes batch element b.

Per-core dataflow (all matmuls bf16 in / fp32 psum accumulate):
  host:  xT = x[b].T (w-major, partition-major relayout), Wqk = [Wq | Wk/8],
         mT = mask.T (consumption-major, 0/1 uint8)
  warm:  ~16 junk matmuls with no data deps run at body start so the PE HAM
         clock-gate flips to 2.4 GHz before the real work arrives (otherwise
         the first ~30us runs at 1.2 GHz).
  DMA:   xt on the SP HWDGE ring (first in line, full bandwidth), weights +
         biases on the ACT HWDGE ring (parallel issue), mask via GpSimd SWDGE
         as uint8 cast to bf16 in-flight (halves mask HBM traffic) in 8 1MB
         chunks that land in consumption order.
  QK:    QK[n,128] = xT.T @ Wqk + bqk (bias via tensor_scalar), cast bf16 ->
         QQ/KK duplicated across partition halves (row-tiled score matmuls)
  V:     V[k,64] = xT.T @ Wv (xt-chunk stationary); bias folded in the DVE
         psum->sbuf copy against a broadcast bv tile; ones column appended so
         the PV matmul also produces the softmax denominator for free
  ST:    ST[k,q] = KK.T @ QQ per 128-k-chunk (K=64 contraction: two chunks run
         concurrently in the PE array via row tiling at partitions 0/64)
  P:     P = exp(ST) on ScalarE (psum -> sbuf bf16), P *= mT (VectorE)
  OT:    OT[65,q] += V'[kchunk].T @ P[kchunk]  (accumulate over 16 k-chunks;
         k outer so each mask chunk is consumed right when it lands)
  out:   OT[65,2048] copied psum->sbuf (VectorE; ScalarE stays exp-only) and
         DMA'd; host does the final out[q,h] = OT[h,q]/OT[64,q] normalization.
"""

import numpy as np
import ml_dtypes

import bass_rust
import concourse.bass as bass
import concourse.mybir as mybir
import concourse.tile as tile
from concourse.bass_utils import run_bass_kernel_spmd

BF16 = ml_dtypes.bfloat16
F32 = mybir.dt.float32
BF = mybir.dt.bfloat16
U8 = mybir.dt.uint8

N_CORES = 8
SEQ = 2048
WIDTH = 768
HEAD = 64
NCH = WIDTH // 128      # 6 contraction chunks for the projections
NKC = SEQ // 128        # 16 key chunks
QT = 1024               # q tile (columns processed per main-loop sweep)
NQT = SEQ // QT
N_WARM = 12             # junk matmuls bridging preamble-end -> first real MM


def _split_excess_waits(nc, max_waits=1):
    """walrus in this container rejects >1 sync wait per instruction; hoist
    extras onto preceding same-engine NoOps (same semantics: the engine
    executes its stream in order, so waiting earlier is equivalent)."""
    n = 0
    for bb in nc.main_func.blocks:
        new_list = []
        for ins in bb.instructions:
            si = ins.sync_info
            if si is not None and len(si.on_wait) > max_waits:
                waits = list(si.on_wait)
                extra, keep = waits[:-max_waits], waits[-max_waits:]
                for j, w in enumerate(extra):
                    nop = bass_rust.InstNoOp(
                        name=f"{ins.name}-ws{j}", engine=ins.engine, ins=[], outs=[]
                    )
                    nop.sync_info = mybir.SyncInfo(on_wait=[w], on_update=[])
                    new_list.append(nop)
                    n += 1
                ins.sync_info = mybir.SyncInfo(
                    on_wait=keep, on_update=list(si.on_update)
                )
            new_list.append(ins)
        bb.instructions = new_list
    return n


def _strip_tail(nc):
    """Drop the NRT pseudo-sync ISA op and the second all-engine barrier that
    TileContext emits after the semaphore reset — ~4-5us of fixed tail. The
    final DMA-drain + first barrier + sem reset are kept, so re-execution of
    the NEFF still starts from clean semaphores."""
    for bb in nc.main_func.blocks:
        ins_list = list(bb.instructions)
        idx = None
        for i, ins in enumerate(ins_list):
            if getattr(ins, "is_reset_sema", False):
                idx = i
        if idx is not None and idx > len(ins_list) - 20:
            bb.instructions = ins_list[:idx + 1]
    return nc


def _build():
    nc = bass.Bass("TRN2", target_bir_lowering=False, debug=False,
                   num_devices=N_CORES)

    # partition-major host layouts: row p holds everything partition p needs,
    # so each DMA is 128 large contiguous descriptors.
    xT_d = nc.declare_dram_parameter("xT", [128, 4 * NCH * 512], BF, False).ap()
    wqk_d = nc.declare_dram_parameter("Wqk", [128, NCH * 128], BF, False).ap()
    wv_d = nc.declare_dram_parameter("Wv", [128, NCH * HEAD], BF, False).ap()
    bqk_d = nc.declare_dram_parameter("bqk", [128, 1], F32, False).ap()
    bvb_d = nc.declare_dram_parameter("bvb", [128, HEAD], BF, False).ap()
    mT_d = nc.declare_dram_parameter("mT", [128, NKC * SEQ], BF, False).ap()
    ot_d = nc.declare_dram_parameter("ot", [HEAD + 1, SEQ], F32, True).ap()

    EXP = mybir.ActivationFunctionType.Exp

    with tile.TileContext(nc) as tc:
        with (
            tc.tile_pool(name="const", bufs=1) as const,
            tc.tile_pool(name="pp", bufs=6) as ppool,
            tc.tile_pool(name="ep", bufs=2) as epool,
            tc.tile_pool(name="stp", bufs=3, space="PSUM") as stp,
            tc.tile_pool(name="otp", bufs=1, space="PSUM") as otp,
        ):
            # ---- PE warm-up: data-independent matmuls fill the PE queue so
            # the HAM clock gate sees ~3.4us of sustained activity and flips
            # to 2.4 GHz right as the first real matmuls arrive. Results are
            # discarded (the pool buffer is recycled by the projections).
            wsrc = const.tile([128, 256], BF)
            nc.vector.memset(wsrc, 0.0)
            warm_ps = stp.tile([128, 256], F32, tag="st", name="warm_ps")
            for i in range(N_WARM):
                nc.tensor.matmul(warm_ps, wsrc[:, 0:128], wsrc[:, :],
                                 start=True, stop=True)

            # ---- inputs into SBUF, all on the single SP HWDGE ring ----
            # Ring FIFO order IS the priority order: xt group 0 first (the
            # whole kernel is gated on it, split in two so the projections
            # can start on chunks c0-c2 while c3-c5 are still in flight),
            # then weights/biases, then mask chunks interleaved between the
            # later xt groups exactly in consumption order.
            xt = const.tile([128, 4, NCH, 512], BF)
            mt = const.tile([128, NQT * NKC, QT], BF)
            wqk = const.tile([128, NCH, 128], BF)
            bqk = const.tile([128, 1], F32)
            wv = const.tile([128, NCH, HEAD], BF)
            bvb = const.tile([128, HEAD], BF)

            def dma_xt(t):
                nc.sync.dma_start(
                    out=xt[:, t, :, :],
                    in_=xT_d[:, t * NCH * 512:(t + 1) * NCH * 512],
                )

            def dma_m(u0, u1):   # mask units [u0, u1) in consumption order
                nc.sync.dma_start(
                    out=mt[:, u0:u1, :], in_=mT_d[:, u0 * QT:u1 * QT],
                )

            nc.sync.dma_start(out=xt[:, 0, 0:3, :], in_=xT_d[:, 0:3 * 512])
            nc.sync.dma_start(out=xt[:, 0, 3:6, :],
                              in_=xT_d[:, 3 * 512:NCH * 512])
            nc.sync.dma_start(out=wqk, in_=wqk_d)
            nc.sync.dma_start(out=bqk, in_=bqk_d)
            nc.sync.dma_start(out=wv, in_=wv_d)
            nc.sync.dma_start(out=bvb, in_=bvb_d)
            dma_m(0, 2)
            dma_xt(1)
            dma_m(2, 4)
            dma_xt(2)
            dma_m(4, 6)
            dma_m(6, 8)
            dma_xt(3)
            dma_m(8, 12)
            dma_m(12, 16)
            dma_m(16, 20)
            dma_m(20, 24)
            dma_m(24, 28)
            dma_m(28, 32)

            # ---- projections, one 512-column group per xT DMA so they
            # pipeline behind the xT arrival; groups t2/t3 are emitted
            # interleaved into the early main loop (their xT lands later) ----
            qktmp = const.tile([128, SEQ], BF)   # Q on parts 0:64, K on 64:128
            qq = const.tile([128, SEQ], BF)      # Q duplicated on both halves
            kk = const.tile([128, SEQ], BF)      # K duplicated on both halves
            vp = const.tile([128, NKC, HEAD + 1], BF)   # V' with ones column

            def proj_qk(t):
                qk_ps = stp.tile([128, 512], F32, tag="st", name=f"qk_ps{t}")
                for c in range(NCH):
                    nc.tensor.matmul(
                        qk_ps, wqk[:, c, :], xt[:, t, c, :],
                        start=(c == 0), stop=(c == NCH - 1),
                    )
                cols = slice(t * 512, (t + 1) * 512)
                nc.vector.tensor_scalar(
                    out=qktmp[:, cols], in0=qk_ps, scalar1=bqk[:, 0:1],
                    scalar2=None, op0=mybir.AluOpType.add,
                )
                # a-half operands (qq/kk rows 0:64) first so st_a can start
                # while the b-half copies still run.
                nc.vector.tensor_copy(out=qq[0:64, cols], in_=qktmp[0:64, cols])
                nc.vector.tensor_copy(out=kk[0:64, cols], in_=qktmp[64:128, cols])
                nc.vector.tensor_copy(out=qq[64:128, cols], in_=qktmp[0:64, cols])
                nc.vector.tensor_copy(out=kk[64:128, cols], in_=qktmp[64:128, cols])

            def proj_v(t, jlo, jhi):
                v_ps = stp.tile([128, jhi - jlo, HEAD], F32, tag="st",
                                name=f"v_ps{t}_{jlo}")
                for j in range(jlo, jhi):
                    for c in range(NCH):
                        nc.tensor.matmul(
                            v_ps[:, j - jlo, :],
                            xt[:, t, c, j * 128:(j + 1) * 128],
                            wv[:, c, :], start=(c == 0), stop=(c == NCH - 1),
                        )
                lo, hi = 4 * t + jlo, 4 * t + jhi
                # psum -> sbuf with the bias folded in (bv broadcast tile);
                # the ones column gives the softmax denominator in the PV mm.
                for j in range(jlo, jhi):
                    nc.vector.tensor_add(
                        out=vp[:, 4 * t + j, 0:HEAD],
                        in0=v_ps[:, j - jlo, :], in1=bvb,
                    )
                nc.vector.memset(vp[:, lo:hi, HEAD:HEAD + 1], 1.0)

            proj_qk(0)
            proj_v(0, 0, 4)
            proj_qk(1)

            # ---- main loop (q outer): scores -> exp -> mask -> PV ----
            # PV for iteration kp-1 is emitted alongside the scores for kp
            # (as a list of matmul specs) so the PE never stalls mid-stream.
            # The first two iterations of q0 are split into 512-wide halves:
            # the h0 half depends only on xT group t0, so exp starts earlier
            # while t1 is still arriving.
            NKP = NKC // 2
            for q in range(NQT):
                ot_ps = otp.tile([HEAD + 1, QT], F32, tag="ot", name=f"ot_ps{q}")
                prev = []
                for kp in range(NKP + 1):
                    cur = []
                    if kp < NKP:
                        k0, k1 = 2 * kp, 2 * kp + 1
                        split = (q == 0 and kp < 2)
                        halves = ((0, 512), (512, 1024)) if split else ((0, QT),)
                        for lo, hi in halves:
                            w = hi - lo
                            st_a = stp.tile([128, w], F32, tag="st",
                                            name=f"st_a{q}_{kp}_{lo}")
                            st_b = stp.tile([128, w], F32, tag="st",
                                            name=f"st_b{q}_{kp}_{lo}")
                            for h in range(w // 512):
                                gq = slice(q * QT + lo + h * 512,
                                           q * QT + lo + (h + 1) * 512)
                                nc.tensor.matmul(
                                    st_a[:, h * 512:(h + 1) * 512],
                                    kk[0:64, k0 * 128:(k0 + 1) * 128],
                                    qq[0:64, gq], start=True, stop=True,
                                )
                                nc.tensor.matmul(
                                    st_b[:, h * 512:(h + 1) * 512],
                                    kk[64:128, k1 * 128:(k1 + 1) * 128],
                                    qq[64:128, gq], start=True, stop=True,
                                )
                            p_a = ppool.tile([128, w], BF, tag="p",
                                             name=f"p_a{q}_{kp}_{lo}")
                            p_b = ppool.tile([128, w], BF, tag="p",
                                             name=f"p_b{q}_{kp}_{lo}")
                            nc.scalar.activation(p_a, st_a, EXP)
                            nc.scalar.activation(p_b, st_b, EXP)
                            mos = slice(lo, hi)
                            nc.vector.tensor_mul(p_a, p_a, mt[:, q * NKC + k0, mos])
                            nc.vector.tensor_mul(p_b, p_b, mt[:, q * NKC + k1, mos])
                            for h in range(w // 512):
                                ohs = slice(lo + h * 512, lo + (h + 1) * 512)
                                phs = slice(h * 512, (h + 1) * 512)
                                cur.append((ohs, k0, p_a, phs))
                                cur.append((ohs, k1, p_b, phs))
                        if q == 0:
                            # feed remaining projection work into the PE
                            # stream in small wedges so ScalarE never starves
                            if kp == 0:
                                proj_v(1, 0, 2)
                            elif kp == 1:
                                proj_v(1, 2, 4)
                                proj_qk(2)
                            elif kp == 2:
                                proj_v(2, 0, 2)
                            elif kp == 3:
                                proj_v(2, 2, 4)
                                proj_qk(3)
                            elif kp == 4:
                                proj_v(3, 0, 2)
                            elif kp == 5:
                                proj_v(3, 2, 4)
                    for ohs, k, p, phs in prev:
                        nc.tensor.matmul(
                            ot_ps[:, ohs], vp[:, k, :], p[:, phs],
                            start=(k == 0), stop=(k == NKC - 1),
                        )
                    prev = cur

                # psum -> sbuf on VectorE (ScalarE stays exp-only), then DMA
                # out on the SP ring (idle once the xt groups are in).
                ot_sb = epool.tile([HEAD + 1, QT], F32, tag="osb", name=f"ot_sb{q}")
                nc.vector.tensor_copy(out=ot_sb, in_=ot_ps)
                nc.sync.dma_start(out=ot_d[:, q * QT:(q + 1) * QT], in_=ot_sb)

    _split_excess_waits(nc)
    _strip_tail(nc)
    return nc


_CACHE = {}


def _get_nc():
    if "nc" not in _CACHE:
        _CACHE["nc"] = _build()
    return _CACHE["nc"]


def _prep_in_maps(x, Wq, bq, Wk, bk, Wv, bv, mask):
    x = np.asarray(x, dtype=np.float32)
    Wqk = np.concatenate(
        [np.asarray(Wq, np.float32), np.asarray(Wk, np.float32) * 0.125], axis=1
    )
    # partition-major: row p holds [c0 cols | c1 cols | ...] for w = c*128+p
    Wqkh = np.ascontiguousarray(
        Wqk.reshape(NCH, 128, 128).transpose(1, 0, 2).reshape(128, NCH * 128)
    ).astype(BF16)
    Wvh = np.ascontiguousarray(
        np.asarray(Wv, np.float32).reshape(NCH, 128, HEAD)
        .transpose(1, 0, 2).reshape(128, NCH * HEAD)
    ).astype(BF16)
    bqk = np.concatenate(
        [np.asarray(bq, np.float32), np.asarray(bk, np.float32) * 0.125]
    ).astype(np.float32).reshape(128, 1)
    bvb = np.broadcast_to(
        np.asarray(bv, np.float32).astype(BF16).reshape(1, HEAD), (128, HEAD)
    ).copy()
    # mTh[p, (h*NKC+c)*QT + j] = mask[h*QT+j, c*128+p]: consumption-major
    # (q-half, then k-chunk).
    mTh = np.ascontiguousarray(
        np.asarray(mask, np.float32).T.reshape(NKC, 128, NQT, QT)
        .transpose(1, 2, 0, 3).reshape(128, NKC * SEQ)
    ).astype(BF16)
    in_maps = []
    for b in range(N_CORES):
        # xth[p, t, c, j] = x[b][t*512+j, c*128+p]
        xth = np.ascontiguousarray(
            x[b].reshape(4, 512, NCH, 128).transpose(3, 0, 2, 1)
            .reshape(128, 4 * NCH * 512)
        ).astype(BF16)
        in_maps.append({
            "xT": xth, "Wqk": Wqkh, "Wv": Wvh, "bqk": bqk, "bvb": bvb,
            "mT": mTh,
        })
    return in_maps


def _run(in_maps, trace=False, **kw):
    nc = _get_nc()
    return run_bass_kernel_spmd(nc, in_maps, list(range(N_CORES)), trace=trace, **kw)


def kernel(x, Wq, bq, Wk, bk, Wv, bv, mask):
    in_maps = _prep_in_maps(x, Wq, bq, Wk, bk, Wv, bv, mask)
    res = _run(in_maps)
    out = np.empty((N_CORES, SEQ, HEAD), np.float32)
    for b in range(N_CORES):
        ot = np.asarray(res.results[b]["ot"])          # [65, 2048] f32
        out[b] = (ot[:HEAD] / ot[HEAD:HEAD + 1]).T     # normalize + transpose
    return out


# revision 15
# speedup vs baseline: 1.3522x; 1.1020x over previous
"""Single-head attention on 8 Trainium2 NeuronCores.

Problem: x[8, 2048, 768], Wq/Wk/Wv[768, 64]+biases, mask[2048, 2048] int32
Output:  softmax(mask(Q K^T / 8)) V   -> [8, 2048, 64] f32

Sharding: data-parallel over batch — core b computes batch element b.

Per-core dataflow (all matmuls bf16 in / fp32 psum accumulate):
  host:  xT = x[b].T (w-major, partition-major relayout), Wqk = [Wq | Wk/8],
         mT = mask.T (consumption-major, 0/1 uint8)
  warm:  ~16 junk matmuls with no data deps run at body start so the PE HAM
         clock-gate flips to 2.4 GHz before the real work arrives (otherwise
         the first ~30us runs at 1.2 GHz).
  DMA:   xt on the SP HWDGE ring (first in line, full bandwidth), weights +
         biases on the ACT HWDGE ring (parallel issue), mask via GpSimd SWDGE
         as uint8 cast to bf16 in-flight (halves mask HBM traffic) in 8 1MB
         chunks that land in consumption order.
  QK:    QK[n,128] = xT.T @ Wqk + bqk (bias via tensor_scalar), cast bf16 ->
         QQ/KK duplicated across partition halves (row-tiled score matmuls)
  V:     V[k,64] = xT.T @ Wv (xt-chunk stationary); bias folded in the DVE
         psum->sbuf copy against a broadcast bv tile; ones column appended so
         the PV matmul also produces the softmax denominator for free
  ST:    ST[k,q] = KK.T @ QQ per 128-k-chunk (K=64 contraction: two chunks run
         concurrently in the PE array via row tiling at partitions 0/64)
  P:     P = exp(ST) on ScalarE (psum -> sbuf bf16), P *= mT (VectorE)
  OT:    OT[65,q] += V'[kchunk].T @ P[kchunk]  (accumulate over 16 k-chunks;
         k outer so each mask chunk is consumed right when it lands)
  out:   OT[65,2048] copied psum->sbuf (VectorE; ScalarE stays exp-only) and
         DMA'd; host does the final out[q,h] = OT[h,q]/OT[64,q] normalization.
"""

import numpy as np
import ml_dtypes

import bass_rust
import concourse.bass as bass
import concourse.mybir as mybir
import concourse.tile as tile
from concourse.bass_utils import run_bass_kernel_spmd

BF16 = ml_dtypes.bfloat16
F32 = mybir.dt.float32
BF = mybir.dt.bfloat16
U8 = mybir.dt.uint8

N_CORES = 8
SEQ = 2048
WIDTH = 768
HEAD = 64
NCH = WIDTH // 128      # 6 contraction chunks for the projections
NKC = SEQ // 128        # 16 key chunks
QT = 1024               # q tile (columns processed per main-loop sweep)
NQT = SEQ // QT
N_WARM = 26             # junk matmuls bridging preamble-end -> first real MM


def _split_excess_waits(nc, max_waits=1):
    """walrus in this container rejects >1 sync wait per instruction; hoist
    extras onto preceding same-engine NoOps (same semantics: the engine
    executes its stream in order, so waiting earlier is equivalent)."""
    n = 0
    for bb in nc.main_func.blocks:
        new_list = []
        for ins in bb.instructions:
            si = ins.sync_info
            if si is not None and len(si.on_wait) > max_waits:
                waits = list(si.on_wait)
                extra, keep = waits[:-max_waits], waits[-max_waits:]
                for j, w in enumerate(extra):
                    nop = bass_rust.InstNoOp(
                        name=f"{ins.name}-ws{j}", engine=ins.engine, ins=[], outs=[]
                    )
                    nop.sync_info = mybir.SyncInfo(on_wait=[w], on_update=[])
                    new_list.append(nop)
                    n += 1
                ins.sync_info = mybir.SyncInfo(
                    on_wait=keep, on_update=list(si.on_update)
                )
            new_list.append(ins)
        bb.instructions = new_list
    return n


def _strip_tail(nc):
    """Drop the NRT pseudo-sync ISA op and the second all-engine barrier that
    TileContext emits after the semaphore reset — ~4-5us of fixed tail. The
    final DMA-drain + first barrier + sem reset are kept, so re-execution of
    the NEFF still starts from clean semaphores."""
    for bb in nc.main_func.blocks:
        ins_list = list(bb.instructions)
        idx = None
        for i, ins in enumerate(ins_list):
            if getattr(ins, "is_reset_sema", False):
                idx = i
        if idx is not None and idx > len(ins_list) - 20:
            bb.instructions = ins_list[:idx + 1]
    return nc


def _build():
    nc = bass.Bass("TRN2", target_bir_lowering=False, debug=False,
                   num_devices=N_CORES)

    # partition-major host layouts: row p holds everything partition p needs,
    # so each DMA is 128 large contiguous descriptors.
    xT_d = nc.declare_dram_parameter("xT", [128, 4 * NCH * 512], BF, False).ap()
    wqk_d = nc.declare_dram_parameter("Wqk", [128, NCH * 128], BF, False).ap()
    wv_d = nc.declare_dram_parameter("Wv", [128, NCH * HEAD], BF, False).ap()
    bqk_d = nc.declare_dram_parameter("bqk", [128, 1], F32, False).ap()
    mT_d = nc.declare_dram_parameter("mT", [128, NKC * SEQ], BF, False).ap()
    ot_d = nc.declare_dram_parameter("ot", [HEAD + 1, SEQ], F32, True).ap()

    EXP = mybir.ActivationFunctionType.Exp

    with tile.TileContext(nc) as tc:
        with (
            tc.tile_pool(name="const", bufs=1) as const,
            tc.tile_pool(name="pp", bufs=6) as ppool,
            tc.tile_pool(name="ep", bufs=2) as epool,
            tc.tile_pool(name="stp", bufs=3, space="PSUM") as stp,
            tc.tile_pool(name="otp", bufs=1, space="PSUM") as otp,
        ):
            # ---- PE warm-up: data-independent matmuls fill the PE queue so
            # the HAM clock gate sees ~3.4us of sustained activity and flips
            # to 2.4 GHz right as the first real matmuls arrive. Results are
            # discarded (the pool buffer is recycled by the projections).
            wsrc = const.tile([128, 256], BF)
            nc.vector.memset(wsrc, 0.0)
            warm_ps = stp.tile([128, 256], F32, tag="st", name="warm_ps")
            for i in range(N_WARM):
                nc.tensor.matmul(warm_ps, wsrc[:, 0:128], wsrc[:, :],
                                 start=True, stop=True)

            # ---- inputs into SBUF, all on the single SP HWDGE ring ----
            # Ring FIFO order IS the priority order: xt group 0 first (the
            # whole kernel is gated on it, split in two so the projections
            # can start on chunks c0-c2 while c3-c5 are still in flight),
            # then weights/biases, then mask chunks interleaved between the
            # later xt groups exactly in consumption order.
            xt = const.tile([128, 4, NCH, 512], BF)
            mt = const.tile([128, NQT * NKC, QT], BF)
            wqk = const.tile([128, NCH, 128], BF)
            bqk = const.tile([128, 1], F32)
            wv = const.tile([128, NCH, HEAD], BF)

            def dma_xt(t):
                nc.sync.dma_start(
                    out=xt[:, t, :, :],
                    in_=xT_d[:, t * NCH * 512:(t + 1) * NCH * 512],
                )

            def dma_m(u0, u1):   # mask units [u0, u1) in consumption order
                nc.sync.dma_start(
                    out=mt[:, u0:u1, :], in_=mT_d[:, u0 * QT:u1 * QT],
                )

            nc.sync.dma_start(out=xt[:, 0, 0:3, :], in_=xT_d[:, 0:3 * 512])
            nc.sync.dma_start(out=xt[:, 0, 3:6, :],
                              in_=xT_d[:, 3 * 512:NCH * 512])
            nc.sync.dma_start(out=wqk, in_=wqk_d)
            nc.sync.dma_start(out=bqk, in_=bqk_d)
            dma_xt(1)
            nc.sync.dma_start(out=wv, in_=wv_d)
            dma_m(0, 2)
            dma_xt(2)
            dma_m(2, 4)
            dma_m(4, 6)
            dma_xt(3)
            dma_m(6, 8)
            dma_m(8, 12)
            dma_m(12, 16)
            dma_m(16, 24)
            dma_m(24, 32)

            # ---- projections, one 512-column group per xT DMA so they
            # pipeline behind the xT arrival; groups t2/t3 are emitted
            # interleaved into the early main loop (their xT lands later) ----
            # qktmp holds Q on partitions 0:64 and K on 64:128 straight from
            # the projection; the score matmuls read it directly for the
            # operands that live on the right partitions (st_a's moving Q,
            # st_b's stationary K) so only the two cross-half duplicates
            # (K down to 0:64, Q up to 64:128) need VectorE copies.
            qktmp = const.tile([128, SEQ], BF)
            kklo = const.tile([64, SEQ], BF)     # K duplicated to parts 0:64
            qqhi = const.tile([128, SEQ], BF)    # Q duplicated to parts 64:128
            vp = const.tile([128, NKC, HEAD + 1], BF)   # V' with ones column

            def proj_qk(t):
                qk_ps = stp.tile([128, 512], F32, tag="st", name=f"qk_ps{t}")
                for c in range(NCH):
                    nc.tensor.matmul(
                        qk_ps, wqk[:, c, :], xt[:, t, c, :],
                        start=(c == 0), stop=(c == NCH - 1),
                    )
                cols = slice(t * 512, (t + 1) * 512)
                nc.vector.tensor_scalar(
                    out=qktmp[:, cols], in0=qk_ps, scalar1=bqk[:, 0:1],
                    scalar2=None, op0=mybir.AluOpType.add,
                )
                nc.vector.tensor_copy(out=kklo[:, cols], in_=qktmp[64:128, cols])
                nc.vector.tensor_copy(out=qqhi[64:128, cols], in_=qktmp[0:64, cols])

            def proj_v(t, jlo, jhi):
                v_ps = stp.tile([128, jhi - jlo, HEAD], F32, tag="st",
                                name=f"v_ps{t}_{jlo}")
                for j in range(jlo, jhi):
                    for c in range(NCH):
                        nc.tensor.matmul(
                            v_ps[:, j - jlo, :],
                            xt[:, t, c, j * 128:(j + 1) * 128],
                            wv[:, c, :], start=(c == 0), stop=(c == NCH - 1),
                        )
                lo, hi = 4 * t + jlo, 4 * t + jhi
                # bv is NOT added here: out = (sum_k P (V+bv))/D = PV/D + bv,
                # so the host adds bv after normalization for free.
                nc.vector.tensor_copy(out=vp[:, lo:hi, 0:HEAD], in_=v_ps)
                nc.vector.memset(vp[:, lo:hi, HEAD:HEAD + 1], 1.0)

            proj_qk(0)
            proj_v(0, 0, 4)
            proj_qk(1)

            # ---- main loop (q outer): scores -> exp -> mask -> PV ----
            # PV for iteration kp-1 is emitted alongside the scores for kp
            # (as a list of matmul specs) so the PE never stalls mid-stream.
            NKP = NKC // 2
            for q in range(NQT):
                ot_ps = otp.tile([HEAD + 1, QT], F32, tag="ot", name=f"ot_ps{q}")
                prev = []
                for kp in range(NKP + 1):
                    cur = []
                    if kp < NKP:
                        k0, k1 = 2 * kp, 2 * kp + 1
                        st_a = stp.tile([128, QT], F32, tag="st",
                                        name=f"st_a{q}_{kp}")
                        st_b = stp.tile([128, QT], F32, tag="st",
                                        name=f"st_b{q}_{kp}")
                        for h in range(QT // 512):
                            gq = slice(q * QT + h * 512, q * QT + (h + 1) * 512)
                            nc.tensor.matmul(
                                st_a[:, h * 512:(h + 1) * 512],
                                kklo[:, k0 * 128:(k0 + 1) * 128],
                                qktmp[0:64, gq], start=True, stop=True,
                            )
                            nc.tensor.matmul(
                                st_b[:, h * 512:(h + 1) * 512],
                                qktmp[64:128, k1 * 128:(k1 + 1) * 128],
                                qqhi[64:128, gq], start=True, stop=True,
                            )
                        p_a = ppool.tile([128, QT], BF, tag="p",
                                         name=f"p_a{q}_{kp}")
                        p_b = ppool.tile([128, QT], BF, tag="p",
                                         name=f"p_b{q}_{kp}")
                        nc.scalar.activation(p_a, st_a, EXP)
                        nc.scalar.activation(p_b, st_b, EXP)
                        nc.vector.tensor_mul(p_a, p_a, mt[:, q * NKC + k0, :])
                        nc.vector.tensor_mul(p_b, p_b, mt[:, q * NKC + k1, :])
                        for h in range(QT // 512):
                            hs = slice(h * 512, (h + 1) * 512)
                            cur.append((hs, k0, p_a))
                            cur.append((hs, k1, p_b))
                        if q == 0:
                            # feed remaining projection work into the PE
                            # stream in small wedges so ScalarE never starves
                            if kp == 0:
                                proj_v(1, 0, 2)
                            elif kp == 1:
                                proj_v(1, 2, 4)
                                proj_qk(2)
                            elif kp == 2:
                                proj_v(2, 0, 2)
                            elif kp == 3:
                                proj_v(2, 2, 4)
                                proj_qk(3)
                            elif kp == 4:
                                proj_v(3, 0, 2)
                            elif kp == 5:
                                proj_v(3, 2, 4)
                    for hs, k, p in prev:
                        nc.tensor.matmul(
                            ot_ps[:, hs], vp[:, k, :], p[:, hs],
                            start=(k == 0), stop=(k == NKC - 1),
                        )
                    prev = cur

                # psum -> sbuf on VectorE (ScalarE stays exp-only), in two
                # halves so the copy doesn't block the DVE queue for 1.2us
                # at the q-tile boundary; DMA out per half on the SP ring.
                ot_sb = epool.tile([HEAD + 1, QT], F32, tag="osb", name=f"ot_sb{q}")
                for h in range(2):
                    hs = slice(h * 512, (h + 1) * 512)
                    nc.vector.tensor_copy(out=ot_sb[:, hs], in_=ot_ps[:, hs])
                    nc.sync.dma_start(
                        out=ot_d[:, q * QT + h * 512:q * QT + (h + 1) * 512],
                        in_=ot_sb[:, hs],
                    )

    _split_excess_waits(nc)
    _strip_tail(nc)
    return nc


_CACHE = {}


def _get_nc():
    if "nc" not in _CACHE:
        _CACHE["nc"] = _build()
    return _CACHE["nc"]


def _prep_in_maps(x, Wq, bq, Wk, bk, Wv, bv, mask):
    x = np.asarray(x, dtype=np.float32)
    Wqk = np.concatenate(
        [np.asarray(Wq, np.float32), np.asarray(Wk, np.float32) * 0.125], axis=1
    )
    # partition-major: row p holds [c0 cols | c1 cols | ...] for w = c*128+p
    Wqkh = np.ascontiguousarray(
        Wqk.reshape(NCH, 128, 128).transpose(1, 0, 2).reshape(128, NCH * 128)
    ).astype(BF16)
    Wvh = np.ascontiguousarray(
        np.asarray(Wv, np.float32).reshape(NCH, 128, HEAD)
        .transpose(1, 0, 2).reshape(128, NCH * HEAD)
    ).astype(BF16)
    bqk = np.concatenate(
        [np.asarray(bq, np.float32), np.asarray(bk, np.float32) * 0.125]
    ).astype(np.float32).reshape(128, 1)
    # mTh[p, (h*NKC+c)*QT + j] = mask[h*QT+j, c*128+p]: consumption-major
    # (q-half, then k-chunk).
    mTh = np.ascontiguousarray(
        np.asarray(mask, np.float32).T.reshape(NKC, 128, NQT, QT)
        .transpose(1, 2, 0, 3).reshape(128, NKC * SEQ)
    ).astype(BF16)
    in_maps = []
    for b in range(N_CORES):
        # xth[p, t, c, j] = x[b][t*512+j, c*128+p]
        xth = np.ascontiguousarray(
            x[b].reshape(4, 512, NCH, 128).transpose(3, 0, 2, 1)
            .reshape(128, 4 * NCH * 512)
        ).astype(BF16)
        in_maps.append({
            "xT": xth, "Wqk": Wqkh, "Wv": Wvh, "bqk": bqk, "mT": mTh,
        })
    return in_maps


def _run(in_maps, trace=False, **kw):
    nc = _get_nc()
    return run_bass_kernel_spmd(nc, in_maps, list(range(N_CORES)), trace=trace, **kw)


def kernel(x, Wq, bq, Wk, bk, Wv, bv, mask):
    in_maps = _prep_in_maps(x, Wq, bq, Wk, bk, Wv, bv, mask)
    res = _run(in_maps)
    bvf = np.asarray(bv, np.float32).reshape(1, HEAD)
    out = np.empty((N_CORES, SEQ, HEAD), np.float32)
    for b in range(N_CORES):
        ot = np.asarray(res.results[b]["ot"])              # [65, 2048] f32
        out[b] = (ot[:HEAD] / ot[HEAD:HEAD + 1]).T + bvf   # normalize + bias
    return out
